# revision 17
# baseline (speedup 1.0000x reference)
"""Trainium2 Bass kernel for nn_Encoder: B=1M samples through
concat(x,c) -> per-j Linear(5,3)+ReLU -> Linear(51,32)+ReLU ->
Linear(32,16)+ReLU -> {Linear(16,3) mu, Linear(16,3) log_var}.

Strategy: pure data parallel over 8 NeuronCores; per core 125000 samples
padded to 126976 (31 blocks x 4096).  The problem is memory-regime but
on TRN2 it is simultaneously PE- and elementwise-bound, so three things
matter:

1. Input dtype float8_e3m4 (fp8 with 4 mantissa bits).  The host packs
   each shard FEATURE-MAJOR [128, 126976] (rows 0-50 = x features, rows
   51-84 = c, rest zero): matmul rhs tiles come straight from the DMA
   with no on-chip transpose, and fp8 halves HBM traffic vs bf16
   (DMA time ~ bytes per partition).  The L1 matmul runs mixed-dtype
   (bf16 stationary x fp8e3 moving) at full PE rate; measured BIT-EXACT
   vs an fp32 host matmul on this data.  End-to-end max scale-relative
   error 1.52e-2 (gate 2e-2); fp8e4m3 inputs measure 2.8e-2 = FAIL, so
   e3m4 is the only workable fp8 flavor here.

2. Software-pipelined instruction EMISSION.  All matmuls are N=512
   passes: L1 per tile (K=96 real rows 0-84, M=64, two tiles packed per
   [128,512] psum), L2 per 2-tile pair (K=116), L3 per 4, heads per 8
   (K=128).  Consumers (bias+relu + the next layer matmul) are emitted
   one-to-two periods late and parity-staggered so every companion
   matmul sits in the PE FIFO next to an L1 of the opposite PSUM-half
   (= opposite PE column-group): the PE overlaps subarray-disjoint
   matmuls (measured 1.7x when banks+colgroups differ, 1.28x same-bank),
   and no cross-engine wait ever blocks the FIFO head.  PE+DMA chain
   measured 74.8us/iter vs 108.5us naive-order.

3. DVE/ACT balance.  bias+relu ops are ~85us/engine total work (Pool
   cannot read PSUM on TRN2, so only 2 engines): h1 on DVE with ~1/14
   of ops on ACT, h2/h3/h4 on ACT; psum pools sized ps1x3/ps2x2/ps3x2/
   ps4x1 banks (=8) to maximize the hottest WAR slack (L1 vs h1 drain).

Output: heads psum [48,512] -> bf16 staging [48,1024] -> DRAM;
host un-permutes (v-group/tile'/col) and splits mu/lv.  For_i-slope
metric ~108-112us/core-iteration (was 137 for the bf16 baseline).
"""
import numpy as np
import ml_dtypes

import concourse.bass as bass
import concourse.mybir as mybir
import concourse.tile as tile
from concourse.bass_utils import run_bass_kernel_spmd

AF = mybir.ActivationFunctionType
ALU = mybir.AluOpType
F32 = mybir.dt.float32
BF16 = mybir.dt.bfloat16
E3 = mybir.dt.float8e3
BF16_NP = ml_dtypes.bfloat16
E3_NP = ml_dtypes.float8_e3m4

N_CORES = 8
B_FULL = 1_000_000
PER_CORE = B_FULL // N_CORES      # 125000
BLK = 4096                        # samples per load block
NBLK = 31                         # load blocks per core
NTOT = BLK * NBLK                 # 126976 padded samples per core
TILES = BLK // 512                # 8 tiles of 512 samples per block
NF = 85                           # feature rows (51 from x, 34 from c)

_OFF_W1, _OFF_W2, _OFF_W3, _OFF_WH = 0, 64, 128, 192
_WCOLS = 240


# --- walrus sync-wait-limit workaround (inlined) -------------------------
# The ISA sync slots allow only one wait per regular instruction, but
# Tile's wait assigner can attach several (tail drain, multi-dep
# consumers, self-loading matmuls). Post-pass: move excess waits onto
# freshly inserted same-engine NoOps placed immediately before the needy
# instruction - identical sync semantics, one wait per instruction.
_ws_ctr = [0]


def _split_excess_waits(nc, max_waits=1):
    for fn in nc.m.functions:
        for bb in fn.blocks:
            insts = bb.instructions
            i = 0
            while i < len(insts):
                inst = insts[i]
                si = inst.sync_info
                if si is None or si.on_wait is None or \
                        len(si.on_wait) <= max_waits:
                    i += 1
                    continue
                waits = list(si.on_wait)
                keep = waits[-max_waits:]
                excess = waits[:-max_waits]
                new_nops = []
                for w in excess:
                    _ws_ctr[0] += 1
                    nop = mybir.InstNoOp(
                        name=f"I-waitsplit-{_ws_ctr[0]}",
                        sync_info=mybir.SyncInfo(on_wait=[w], on_update=[]),
                        bass_nofuse=True,
                        engine=inst.engine,
                    )
                    new_nops.append(nop)
                inst.sync_info = mybir.SyncInfo(
                    on_wait=keep, on_update=list(si.on_update or []))
                for j, nop in enumerate(new_nops):
                    insts.insert(i + j, nop)
                i += len(new_nops) + 1



def _host_packs(W1, b1, W2, b2, W3, b3, Wmu, bmu, Wlv, blv):
    """Block-diagonal bf16 weights [128, 240] + fp32 bias pack [128, 4]."""
    W1blk = np.zeros((128, 64), np.float32)
    for j in range(17):
        for o in range(3):
            for k in range(3):
                W1blk[3 * j + k, 3 * j + o] = W1[o, k]
            for k in range(2):
                W1blk[51 + 2 * j + k, 3 * j + o] = W1[o, 3 + k]
    W2blk = np.zeros((128, 64), np.float32)
    W2blk[0:51, 0:32] = W2.T
    W2blk[64:115, 32:64] = W2.T
    W3blk = np.zeros((128, 64), np.float32)
    for t in range(4):
        W3blk[32 * t:32 * t + 32, 16 * t:16 * t + 16] = W3.T
    Wh = np.concatenate([Wmu, Wlv], axis=0)          # [6, 16]
    Whblk = np.zeros((128, 48), np.float32)
    for t in range(8):
        Whblk[16 * t:16 * t + 16, 6 * t:6 * t + 6] = Wh.T
    wpack = np.concatenate([W1blk, W2blk, W3blk, Whblk],
                           axis=1).astype(BF16_NP)   # [128, 240]

    b1v = np.zeros((128,), np.float32)
    for j in range(17):
        for o in range(3):
            b1v[3 * j + o] = b1[o]
            b1v[64 + 3 * j + o] = b1[o]
    b2v = np.tile(b2, 4).astype(np.float32)
    b3v = np.tile(b3, 8).astype(np.float32)
    bh = np.concatenate([bmu, blv])
    bhv = np.zeros((128,), np.float32)
    bhv[0:48] = np.tile(bh, 8)
    bpack = np.stack([b1v, b2v, b3v, bhv], axis=1)   # [128, 4]
    return wpack, bpack


def _prep_core(x_flat, c_flat):
    """[n, 51] + [n, 34] fp32 -> feature-major bf16 [128, NTOT].

    Rows 85-127 stay zero: 128-partition DMA destinations run ~2.5x
    faster than 85-row ones (measured 383 vs 150 GB/s), which more than
    pays for the 50% pad bytes; the L1 weights for rows 85-127 are 0."""
    n = x_flat.shape[0]
    xct = np.zeros((128, NTOT), E3_NP)
    xct[0:51, :n] = x_flat.T
    xct[51:85, :n] = c_flat.T
    return xct


def _unpack_out(out_dev):
    """Device layout [48, NBLK*512] bf16 -> (mu, lv) [NTOT, 3] fp32."""
    out_dev = np.asarray(out_dev, dtype=np.float32)
    arr = out_dev.reshape(8, 6, NBLK, 512)           # [t', o, v, c]
    arr = np.transpose(arr, (2, 0, 3, 1))            # [v, t', c, o]
    arr = arr.reshape(NTOT, 6)
    return arr[:, 0:3], arr[:, 3:6]


def build_kernel(nblk=NBLK, repeat=1, mode='full', unroll=False):
    ntot = BLK * nblk
    nc = bass.Bass("TRN2")
    xcd = nc.dram_tensor("xct", [128, ntot], E3, kind="ExternalInput")
    wd = nc.dram_tensor("wpack", [128, _WCOLS], BF16, kind="ExternalInput")
    bd = nc.dram_tensor("bpack", [128, 4], F32, kind="ExternalInput")
    od = nc.dram_tensor("out_dev", [48, nblk * 512], BF16,
                        kind="ExternalOutput")

    with tile.TileContext(nc) as tc:
        with tc.tile_pool(name="const", bufs=1) as constp, \
             tc.tile_pool(name="xc", bufs=4) as xcp, \
             tc.tile_pool(name="h1", bufs=4) as h1p, \
             tc.tile_pool(name="h2", bufs=3) as h2p, \
             tc.tile_pool(name="h3", bufs=3) as h3p, \
             tc.tile_pool(name="h4", bufs=2) as h4p, \
             tc.tile_pool(name="ps1", bufs=3, space="PSUM") as ps1p, \
             tc.tile_pool(name="ps2", bufs=2, space="PSUM") as ps2p, \
             tc.tile_pool(name="ps3", bufs=2, space="PSUM") as ps3p, \
             tc.tile_pool(name="ps4", bufs=1, space="PSUM") as ps4p:

            wt = constp.tile([128, _WCOLS], BF16)
            bt = constp.tile([128, 4], F32)
            nc.sync.dma_start(out=wt, in_=wd[:, :])
            nc.sync.dma_start(out=bt, in_=bd[:, :])
            w1 = wt[:, _OFF_W1:_OFF_W1 + 64]
            w2 = wt[:, _OFF_W2:_OFF_W2 + 64]
            w3 = wt[:, _OFF_W3:_OFF_W3 + 64]
            wh = wt[:, _OFF_WH:_OFF_WH + 48]
            b1v = bt[:, 0:1]
            b2v = bt[:, 1:2]
            b3v = bt[:, 2:3]
            bhv = bt[0:48, 3:4]

            def _body_flat():
                # Software-pipelined emission: consumers (bias/relu + next
                # matmul) are issued one period late so the PE instruction
                # FIFO always has independent matmuls at its head (avoids
                # head-of-line blocking on cross-engine waits).
                ps1 = {}
                ps2 = {}
                ps3 = {}
                h1s = {}
                h2s = {}
                h3s = {}
                h4buf = [None]
                xcb_cur = [None]
                ntiles = nblk * TILES

                def emit_l1(gt):
                    if gt % TILES == 0:
                        xcb = xcp.tile([128, BLK], E3, name='xcbt')
                        xcb_cur[0] = xcb
                        if mode != 'nodma':
                            b = gt // TILES
                            nc.sync.dma_start(
                                out=xcb,
                                in_=xcd[:, b * BLK:(b + 1) * BLK])
                        else:
                            nc.vector.memset(
                                xcb_cur[0][:, 0:4].bitcast(mybir.dt.uint32),
                                0)
                    t = gt % TILES
                    p = gt // 2
                    half = (gt % 2) * 64
                    if gt % 2 == 0:
                        ps1[p] = ps1p.tile([128, 512], F32, name='ps1t')
                    rhs = xcb_cur[0][0:96, 512 * t:512 * (t + 1)]
                    nc.tensor.matmul(ps1[p][half:half + 64, :],
                                     w1[0:96, :], rhs, start=True, stop=True)

                def emit_h1_l2(q):
                    h1 = h1p.tile([128, 512], BF16, name='h1t')
                    if mode == 'peonly':
                        nc.vector.memset(h1[:, 0:2].bitcast(mybir.dt.uint32),
                                         0)
                    elif q % 14 == 13:
                        # DVE/ACT load balance: ACT takes ~1/14 of h1 ops
                        nc.scalar.activation(h1, ps1.pop(q), AF.Relu,
                                             bias=b1v)
                    else:
                        nc.vector.tensor_scalar(
                            out=h1, in0=ps1.pop(q), scalar1=b1v, scalar2=0.0,
                            op0=ALU.add, op1=ALU.max)
                    h1s[q] = h1
                    u = q // 2
                    half = (q % 2) * 64
                    if q % 2 == 0:
                        ps2[u] = ps2p.tile([128, 512], F32, name='ps2t')
                    nc.tensor.matmul(ps2[u][half:half + 64, :],
                                     w2[0:116, :], h1[0:116, :],
                                     start=True, stop=True)
                    h1s.pop(q - 1, None)

                def emit_h2_l3(u):
                    h2 = h2p.tile([128, 512], BF16, name='h2t')
                    if mode == 'peonly':
                        nc.vector.memset(h2[:, 0:2].bitcast(mybir.dt.uint32),
                                         0)
                    else:
                        nc.scalar.activation(h2, ps2.pop(u), AF.Relu,
                                             bias=b2v)
                    h2s[u] = h2
                    v = u // 2
                    half = (u % 2) * 64
                    if u % 2 == 0:
                        ps3[v] = ps3p.tile([128, 512], F32, name='ps3t')
                    nc.tensor.matmul(ps3[v][half:half + 64, :], w3, h2,
                                     start=True, stop=True)
                    h2s.pop(u - 1, None)

                def emit_h3_heads(v):
                    h3 = h3p.tile([128, 512], BF16, name='h3t')
                    if mode == 'peonly':
                        nc.vector.memset(h3[:, 0:2].bitcast(mybir.dt.uint32),
                                         0)
                    else:
                        nc.scalar.activation(h3, ps3.pop(v), AF.Relu,
                                             bias=b3v)
                    h3s[v] = h3
                    psum4 = ps4p.tile([48, 512], F32, name='ps4t')
                    nc.tensor.matmul(psum4, wh, h3, start=True, stop=True)
                    if v % 2 == 0:
                        h4buf[0] = h4p.tile([48, 1024], BF16, name='h4t')
                    h4t = h4buf[0]
                    off = (v % 2) * 512
                    if mode == 'peonly':
                        nc.vector.memset(
                            h4t[:, off:off + 2].bitcast(mybir.dt.uint16), 0)
                    else:
                        nc.scalar.activation(h4t[:, off:off + 512], psum4,
                                             AF.Identity, bias=bhv)
                        if v % 2 == 1:
                            nc.sync.dma_start(
                                out=od[:, (v - 1) * 512:(v + 1) * 512],
                                in_=h4t)
                        elif v == nblk - 1:
                            nc.sync.dma_start(
                                out=od[:, v * 512:(v + 1) * 512],
                                in_=h4t[:, 0:512])
                    h3s.pop(v - 1, None)

                # Companion emission points are parity-staggered so each
                # L2/L3 matmul (psum half -> col group) lands adjacent to
                # an L1 of the opposite col group: even pairs (cg-low)
                # after an odd-gt L1 (cg-high) and vice versa.  ps1 bufs=3
                # makes the extra one-tile delay on odd pairs WAR-safe.
                for gt in range(ntiles):
                    emit_l1(gt)
                    if gt % 4 == 3:
                        emit_h1_l2((gt - 3) // 2)
                    if gt % 4 == 2 and gt >= 6:
                        emit_h1_l2((gt - 4) // 2)
                    if gt % 8 == 7:
                        emit_h2_l3((gt - 7) // 4)
                    if gt % 8 == 4 and gt >= 12:
                        emit_h2_l3((gt - 8) // 4)
                    if gt % 8 == 7 and gt >= 15:
                        emit_h3_heads(gt // 8 - 1)
                emit_h1_l2(ntiles // 2 - 1)
                emit_h2_l3(ntiles // 4 - 1)
                emit_h3_heads(ntiles // 8 - 1)

            def _body():
              if mode in ('full', 'nodma', 'peonly'):
                  _body_flat()
                  return
              psum1 = psum2 = psum3 = None
              h4buf = None
              h4off = 0
              for b in range(nblk):
                xcb = xcp.tile([128, BLK], E3)
                if mode != 'nodma':
                    qw = BLK // 4
                    for q4 in range(4):
                        nc.sync.dma_start(
                            out=xcb[:, q4 * qw:(q4 + 1) * qw],
                            in_=xcd[:, b * BLK + q4 * qw:
                                    b * BLK + (q4 + 1) * qw])
                else:
                    nc.vector.memset(xcb[:, 0:4].bitcast(mybir.dt.uint32), 0)
                s2 = min(2, nblk)
                if b % s2 == 0:
                    h4buf = h4p.tile([48, 1024 * s2], BF16)
                h4off = (b % s2) * 1024
                if mode == 'peonly':
                    nc.vector.memset(
                        h4buf[:, h4off:h4off + 2].bitcast(mybir.dt.uint16), 0)

                if mode == 'dmaonly':
                    nc.vector.tensor_copy(h4buf[:, h4off:h4off + 512],
                                          xcb[0:48, 0:512])
                    if b % s2 == s2 - 1:
                        oo = ((b - s2 + 1) * 1024) % (nblk * 512 - 2048)
                        nc.sync.dma_start(
                            out=od[:, oo:oo + 2048],
                            in_=h4buf)
                    continue
                if mode == 'inonly':
                    nc.vector.tensor_copy(h4buf[:, h4off:h4off + 2],
                                          xcb[0:48, 0:2])
                    continue
                if mode == 'in128':
                    flat = xcd.rearrange("f (a c) -> (f a) c", c=2048)
                    nrows = NF * (ntot // 2048)
                    for i3 in range(3):
                        st = (128 * (3 * b + i3)) % (nrows - 128)
                        xcb128 = xcp.tile([128, 2048], E3, tag="xcb128")
                        nc.sync.dma_start(out=xcb128,
                                          in_=flat[st:st + 128, :])
                        nc.vector.tensor_copy(h4buf[:, 2 * i3:2 * i3 + 2],
                                              xcb128[0:48, 0:2])
                    continue
                if mode == 'inbig':
                    if b % 2 == 0:
                        xcb2 = xcp.tile([NF, 2 * BLK], E3, tag="xcb2")
                        nc.sync.dma_start(
                            out=xcb2,
                            in_=xcd[:, b * BLK:(b + 2) * BLK])
                        nc.vector.tensor_copy(h4buf[:, 0:2], xcb2[0:48, 0:2])
                    continue
                if mode in ('pe2b', 'pe2c'):
                    # pe2b: pair writes halves of the SAME psum tile/bank.
                    # pe2c: separate banks but K=128 (all row groups).
                    for t in range(TILES):
                        psA = ps1p.tile([128, 512], F32, tag="psA")
                        if mode == 'pe2b':
                            rhsA = xcb[0:96, 512 * t:512 * (t + 1)]
                            nc.tensor.matmul(psA[0:64, :], w1[0:96, 0:64],
                                             rhsA, start=True, stop=True)
                            nc.tensor.matmul(psA[64:128, :], wt[0:96, 0:64],
                                             rhsA, start=True, stop=True)
                        else:
                            psB = ps2p.tile([128, 512], F32, tag="psB")
                            rhsA = xcb[:, 512 * t:512 * (t + 1)]
                            nc.tensor.matmul(psA[0:64, :], w1[:, 0:64],
                                             rhsA, start=True, stop=True)
                            nc.tensor.matmul(psB[64:128, :], wt[:, 0:64],
                                             rhsA, start=True, stop=True)
                    continue
                if mode in ('pe2', 'pe2s'):
                    # timing-only: 32 matmuls/blk in pairs; 'pe2' pairs are
                    # subarray-disjoint (rg012+cg01 vs rg3+cg23), 'pe2s'
                    # pairs share row groups (expect serial).
                    for t in range(TILES):
                        psA = ps1p.tile([128, 512], F32, tag="psA")
                        psB = ps2p.tile([128, 512], F32, tag="psB")
                        rhsA = xcb[0:96, 512 * t:512 * (t + 1)]
                        nc.tensor.matmul(psA[0:64, :], w1[0:96, 0:64], rhsA,
                                         start=True, stop=True)
                        if mode == 'pe2':
                            rhsB = xcb[96:128, 512 * t:512 * (t + 1)]
                            nc.tensor.matmul(psB[64:128, :],
                                             wt[96:128, 0:64], rhsB,
                                             start=True, stop=True,
                                             tile_position=(96, 64))
                        else:
                            rhsB = xcb[0:96, 512 * t:512 * (t + 1)]
                            nc.tensor.matmul(psB[64:128, :],
                                             wt[0:96, 0:64], rhsB,
                                             start=True, stop=True)
                    continue
                for t in range(TILES):
                    rhs0 = xcb[:, 512 * t:512 * (t + 1)]
                    half = (t % 2) * 64
                    if t % 2 == 0:
                        psum1 = ps1p.tile([128, 512], F32)
                    nc.tensor.matmul(psum1[half:half + 64, :], w1, rhs0,
                                     start=True, stop=True)
                    if t % 2 != 1:
                        continue
                    h1 = h1p.tile([128, 512], BF16)
                    if mode == 'peonly':
                        nc.vector.memset(h1[:, 0:2].bitcast(mybir.dt.uint32), 0)
                    else:
                        nc.vector.tensor_scalar(
                            out=h1, in0=psum1, scalar1=b1v, scalar2=0.0,
                            op0=ALU.add, op1=ALU.max)

                    u = t // 2
                    half = (u % 2) * 64
                    if u % 2 == 0:
                        psum2 = ps2p.tile([128, 512], F32)
                    nc.tensor.matmul(psum2[half:half + 64, :], w2, h1,
                                     start=True, stop=True)
                    if u % 2 != 1:
                        continue
                    h2 = h2p.tile([128, 512], BF16)
                    if mode == 'peonly':
                        nc.vector.memset(h2[:, 0:2].bitcast(mybir.dt.uint32), 0)
                    else:
                        nc.scalar.activation(h2, psum2, AF.Relu, bias=b2v)

                    v = t // 4
                    half = (v % 2) * 64
                    if v % 2 == 0:
                        psum3 = ps3p.tile([128, 512], F32)
                    nc.tensor.matmul(psum3[half:half + 64, :], w3, h2,
                                     start=True, stop=True)
                    if v % 2 != 1:
                        continue
                    h3 = h3p.tile([128, 512], BF16)
                    if mode == 'peonly':
                        nc.vector.memset(h3[:, 0:2].bitcast(mybir.dt.uint32), 0)
                    else:
                        nc.scalar.activation(h3, psum3, AF.Relu, bias=b3v)

                    g = t // 8
                    psum4 = ps4p.tile([48, 512], F32)
                    nc.tensor.matmul(psum4, wh, h3, start=True, stop=True)
                    if mode != 'peonly':
                        nc.scalar.activation(
                            h4buf[:, h4off + 512 * g:h4off + 512 * (g + 1)],
                            psum4, AF.Identity, bias=bhv)

                if b % s2 == s2 - 1 and mode not in ('dmaonly', 'inonly'):
                    oo = ((b - s2 + 1) * 1024) % (nblk * 512 - 2048)
                    nc.sync.dma_start(
                        out=od[:, oo:oo + 2048],
                        in_=h4buf)

            if unroll and repeat > 1:
                for _ in range(repeat):
                    _body()
            elif repeat > 1:
                with tc.For_i(0, repeat):
                    _body()
            else:
                _body()

    _split_excess_waits(nc)
    return nc


_NC_CACHE = {}


def _get_nc(nblk=NBLK, repeat=1, mode='full'):
    key = (nblk, repeat, mode)
    if key not in _NC_CACHE:
        _NC_CACHE[key] = build_kernel(nblk, repeat, mode)
    return _NC_CACHE[key]


def kernel(x, c, W1, b1, W2, b2, W3, b3, Wmu, bmu, Wlv, blv, _trace=False):
    x = np.asarray(x, np.float32).reshape(B_FULL, 51)
    c = np.asarray(c, np.float32).reshape(B_FULL, 34)
    wpack, bpack = _host_packs(
        np.asarray(W1, np.float32), np.asarray(b1, np.float32),
        np.asarray(W2, np.float32), np.asarray(b2, np.float32),
        np.asarray(W3, np.float32), np.asarray(b3, np.float32),
        np.asarray(Wmu, np.float32), np.asarray(bmu, np.float32),
        np.asarray(Wlv, np.float32), np.asarray(blv, np.float32))

    in_maps = []
    for core in range(N_CORES):
        sl = slice(core * PER_CORE, (core + 1) * PER_CORE)
        in_maps.append({"xct": _prep_core(x[sl], c[sl]),
                        "wpack": wpack, "bpack": bpack})

    nc = _get_nc()
    res = run_bass_kernel_spmd(nc, in_maps, core_ids=list(range(N_CORES)),
                               trace=_trace)
    mus, lvs = [], []
    for i in range(N_CORES):
        mu_i, lv_i = _unpack_out(res.results[i]["out_dev"])
        mus.append(mu_i[:PER_CORE])
        lvs.append(lv_i[:PER_CORE])
    out = (np.concatenate(mus), np.concatenate(lvs))
    if _trace:
        return out, res
    return out

